# revision 1
# baseline (speedup 1.0000x reference)
"""DYSPN attention-conv kernel for Trainium2 (8 NeuronCores, batch-parallel).

Math (derived from the reference; unfold/fold pair collapses algebraically):
  per image, per tap k=(i,j) != center, ring r = INDEX[i,j], dy = 3-i, dx = 3-j:
    z_k[y,x]  = att_r[y,x] * aff_k[y,x]
    U[y,x]    = sum_k z_k[y,x]                       (S_ppt - att3)
    A[y,x]    = sum_k |z_k[y,x]|                     (S_prime - att3; att >= 0)
    T[y,x]    = sum_k z_k[y+dy, x+dx]  (in-image)    (fold7(z))
  out = r * ((T+att3)*cs - (U+att3)*coarse) + coarse,  r = 1/(A+att3+eps)

Mapping: batch 16 -> 2 images/core. Per image, two 128-row blocks.
  - DVE: z = att*aff (ring-broadcast tensor_tensor, in-place, fp32r out)
  - ACT: |z| (Abs activation, fp32r out)
  - PE : U/A/T reductions as banded-"identity" fp32r matmuls accumulating in
         PSUM; row shifts via diagonal offsets of a band matrix, column shifts
         via guard-band PSUM columns.
  - DVE: epilogue (reciprocal + 5 tensor ops), DMA out.
"""
import sys

sys.path.insert(0, "/opt/trn_rl_repo")

import numpy as np

import concourse.bass as bass  # noqa: F401  (registers engines)
import concourse.tile as tile
from concourse import bacc, mybir
from concourse.bass_utils import run_bass_kernel_spmd

FP32 = mybir.dt.float32
FP32R = mybir.dt.float32r

N_CORES = 8
B_FULL = 16
B_CORE = B_FULL // N_CORES  # 2 images per core
H = W = 256
K = 7
NTAP = 48                 # 49 minus center
BANDW = 390               # band[p, q] = 1 iff q == p + 131
C0 = 131                  # unshifted-identity diagonal column offset
GW = 4                    # zero guard columns on each side of a z tap plane
CHUNKS = [(0, 4), (4, 12), (12, 16), (16, 24), (24, 32), (32, 40), (40, 48)]
TAPS_PER_TILE = 16        # z tiles hold 16 taps; 3 tiles per block
EPS = 1e-6

# ring index of each tap in the 7x7 window (center marked 3, excluded)
_INDEX = np.array([0, 0, 0, 0, 0, 0, 0,
                   0, 1, 1, 1, 1, 1, 0,
                   0, 1, 2, 2, 2, 1, 0,
                   0, 1, 2, 3, 2, 1, 0,
                   0, 1, 2, 2, 2, 1, 0,
                   0, 1, 1, 1, 1, 1, 0,
                   0, 0, 0, 0, 0, 0, 0], dtype=np.int64).reshape(7, 7)

# taps in DRAM order, skipping the center; t = SBUF slot, k = DRAM plane
TAPS = []  # (t, k, ring, dy, dx)
for i in range(K):
    for j in range(K):
        if i == 3 and j == 3:
            continue
        k = i * K + j
        t = k if k < 24 else k - 1
        TAPS.append((t, k, int(_INDEX[i, j]), 3 - i, 3 - j))

# maximal runs of taps (in t-order) sharing one ring -> one DVE mul each
RUNS = []  # (t_lo, t_hi, ring)
for t, k, r, dy, dx in TAPS:
    if RUNS and RUNS[-1][2] == r and RUNS[-1][1] == t:
        RUNS[-1][1] = t + 1
    else:
        RUNS.append([t, t + 1, r])
RUNS = [tuple(x) for x in RUNS]


def _band_matrix() -> np.ndarray:
    band = np.zeros((128, BANDW), dtype=np.float32)
    for p in range(128):
        band[p, p + C0] = 1.0
    return band


def _build():
    nc = bacc.Bacc("TRN2", target_bir_lowering=False, debug=False,
                   num_devices=N_CORES)
    # float32r: same 4-byte layout as fp32 (numpy sees float32). Static BIR
    # typing only — the in-place z multiply re-produces every element rounded
    # to fp32r before any matmul consumes it.
    aff = nc.dram_tensor("affinity", [B_CORE, 49, H, W], FP32R,
                         kind="ExternalInput").ap()
    att = nc.dram_tensor("attention", [B_CORE, 4, H, W], FP32,
                         kind="ExternalInput").ap()
    cs = nc.dram_tensor("current_segmentation", [B_CORE, 1, H, W], FP32,
                        kind="ExternalInput").ap()
    co = nc.dram_tensor("coarse_segmentation", [B_CORE, 1, H, W], FP32,
                        kind="ExternalInput").ap()
    band = nc.dram_tensor("band", [128, BANDW], FP32, kind="ExternalInput").ap()
    out = nc.dram_tensor("out", [B_CORE, 1, H, W], FP32,
                         kind="ExternalOutput").ap()

    with tile.TileContext(nc) as tc:
        with tc.tile_pool(name="const", bufs=1) as cpool, \
             tc.tile_pool(name="zp", bufs=9) as zpool, \
             tc.tile_pool(name="azp", bufs=2) as azpool, \
             tc.tile_pool(name="inp", bufs=2) as ipool, \
             tc.tile_pool(name="ep", bufs=2) as epool, \
             tc.tile_pool(name="ps", bufs=2, space="PSUM") as pspool:

            bandf = cpool.tile([128, BANDW], FP32)
            nc.sync.dma_start(out=bandf[:], in_=band[:, :])
            bandr = cpool.tile([128, BANDW], FP32R)
            nc.vector.tensor_copy(bandr[:], bandf[:])

            for img in range(B_CORE):
                # zts[b][h]: half-block z tiles, 24 taps each, guarded planes.
                # Guards (GW=4 zero cols per side) keep column-shifted matmul
                # reads in-plane with exact zeros at image edges while the
                # PSUM dst stays at [0:256) (fp32r dst restriction).
                zts = [[None] * 3, [None] * 3]
                attfs = []
                csts = []
                cots = []
                att3rs = []
                for b in range(2):
                    r0 = b * 128
                    attf = ipool.tile([128, 4, W], FP32, tag="attf")
                    att_eng = nc.sync if img == 0 else nc.gpsimd
                    att_eng.dma_start(
                        out=attf[:],
                        in_=att[img, :, r0:r0 + 128, :].transpose([1, 0, 2]))
                    cst = ipool.tile([128, W], FP32, tag="cst")
                    nc.gpsimd.dma_start(out=cst[:], in_=cs[img, 0, r0:r0 + 128, :])
                    cot = ipool.tile([128, W], FP32, tag="cot")
                    nc.gpsimd.dma_start(out=cot[:], in_=co[img, 0, r0:r0 + 128, :])
                    att3r = ipool.tile([128, W], FP32R, tag="att3r")
                    nc.scalar.activation(att3r[:], attf[:, 3, :],
                                         mybir.ActivationFunctionType.Copy)
                    attfs.append(attf)
                    csts.append(cst)
                    cots.append(cot)
                    att3rs.append(att3r)
                    for h in range(3):
                        zt = zpool.tile([128, TAPS_PER_TILE, W + 2 * GW],
                                        FP32R, tag="zt")
                        zsrc = bandf[:, 0:1].unsqueeze(1).broadcast_to(
                            [128, TAPS_PER_TILE, GW])
                        nc.scalar.activation(zt[:, :, 0:GW], zsrc,
                                             mybir.ActivationFunctionType.Copy)
                        nc.scalar.activation(zt[:, :, GW + W:], zsrc,
                                             mybir.ActivationFunctionType.Copy)
                        zts[b][h] = zt

                def zslice(b, lo, hi, c0, c1):
                    tp = TAPS_PER_TILE
                    h, l, r2 = lo // tp, lo % tp, (hi - 1) % tp + 1
                    assert (hi - 1) // tp == h
                    return zts[b][h][:, l:r2, c0:c1]

                # graded affinity chunk DMAs (first ones small so compute
                # starts early), all on the sync HWDGE ring; chunk = taps
                # [lo,hi) = a contiguous DRAM plane range (center skipped)
                for ci, (lo, hi) in enumerate(CHUNKS):
                    k_lo = lo if lo < 24 else lo + 1
                    for b in range(2):
                        r0 = b * 128
                        nc.sync.dma_start(
                            out=zslice(b, lo, hi, GW, GW + W),
                            in_=aff[img, k_lo:k_lo + hi - lo,
                                    r0:r0 + 128, :].transpose([1, 0, 2]))

                # ---- per-chunk compute, both blocks interleaved ----
                ident = bandr[:, C0:C0 + 128]
                psU = []
                psA = []
                psT = []
                for b in range(2):
                    pU = pspool.tile([128, W], FP32, tag="U")
                    pA = pspool.tile([128, W], FP32, tag="A")
                    pT = pspool.tile([128, W], FP32, tag="T")
                    psU.append(pU)
                    psA.append(pA)
                    psT.append(pT)

                for ci, (t_lo, t_hi) in enumerate(CHUNKS):
                    # z = att_r * aff (DVE, in-place, fp32r out)
                    for b in range(2):
                        for lo, hi, r in RUNS:
                            lo, hi = max(lo, t_lo), min(hi, t_hi)
                            if lo >= hi:
                                continue
                            zs = zslice(b, lo, hi, GW, GW + W)
                            nc.vector.tensor_tensor(
                                out=zs,
                                in0=zs.bitcast(FP32),
                                in1=attfs[b][:, r:r + 1, :].broadcast_to(
                                    [128, hi - lo, W]),
                                op=mybir.AluOpType.mult)
                    # |z| (ACT), U/A matmuls
                    for b in range(2):
                        for s_lo in range(t_lo, t_hi, 12):
                            s_hi = min(s_lo + 12, t_hi)
                            azt = azpool.tile([128, 12, W], FP32R, tag="azt")
                            nc.scalar.activation(
                                azt[:, 0:s_hi - s_lo, :],
                                zslice(b, s_lo, s_hi, GW, GW + W),
                                mybir.ActivationFunctionType.Abs)
                            for t in range(s_lo, s_hi):
                                nc.tensor.matmul(
                                    out=psU[b][:], lhsT=ident,
                                    rhs=zslice(b, t, t + 1,
                                               GW, GW + W).squeeze(1),
                                    start=(t == 0), stop=False)
                            for tt in range(s_hi - s_lo):
                                nc.tensor.matmul(
                                    out=psA[b][:], lhsT=ident,
                                    rhs=azt[:, tt, :],
                                    start=(s_lo == 0 and tt == 0),
                                    stop=False)
                    # T matmuls: row shift dy via band diagonal offset, col
                    # shift dx via moving-side offset into the guarded plane
                    for b in range(2):
                        for t, k, r, dy, dx in TAPS[t_lo:t_hi]:
                            nc.tensor.matmul(
                                out=psT[b][:],
                                lhsT=bandr[:, C0 + dy:C0 + dy + 128],
                                rhs=zslice(b, t, t + 1,
                                           GW + dx, GW + dx + W).squeeze(1),
                                start=(t == t_lo and ci == 0), stop=False)
                        # cross-block halo rows: full-partition moving; the
                        # band window zeroes all but the halo rows
                        for t, k, r, dy, dx in TAPS[t_lo:t_hi]:
                            if b == 0 and dy > 0:
                                nc.tensor.matmul(
                                    out=psT[0][:],
                                    lhsT=bandr[:, 3 + dy:3 + dy + 128],
                                    rhs=zslice(1, t, t + 1,
                                               GW + dx, GW + dx + W).squeeze(1),
                                    start=False, stop=False)
                            elif b == 1 and dy < 0:
                                nc.tensor.matmul(
                                    out=psT[1][:],
                                    lhsT=bandr[:, 259 + dy:259 + dy + 128],
                                    rhs=zslice(0, t, t + 1,
                                               GW + dx, GW + dx + W).squeeze(1),
                                    start=False, stop=False)

                # close the accumulation groups with the +att3 taps
                for b in range(2):
                    nc.tensor.matmul(out=psU[b][:], lhsT=ident,
                                     rhs=att3rs[b][:], start=False, stop=True)
                    nc.tensor.matmul(out=psA[b][:], lhsT=ident,
                                     rhs=att3rs[b][:], start=False, stop=True)
                    nc.tensor.matmul(out=psT[b][:], lhsT=ident,
                                     rhs=att3rs[b][:], start=False, stop=True)

                # ---- epilogue ----
                for b in range(2):
                    r0 = b * 128
                    e = epool.tile([128, W], FP32, tag="e")
                    nc.vector.tensor_scalar_add(e[:], psA[b][:], EPS)
                    rcp = epool.tile([128, W], FP32, tag="rcp")
                    nc.vector.reciprocal(rcp[:], e[:])
                    m1 = epool.tile([128, W], FP32, tag="m1")
                    nc.vector.tensor_mul(m1[:], psT[b][:], csts[b][:])
                    m2 = epool.tile([128, W], FP32, tag="m2")
                    nc.vector.tensor_mul(m2[:], psU[b][:], cots[b][:])
                    nc.vector.tensor_sub(m1[:], m1[:], m2[:])
                    nc.vector.tensor_mul(m1[:], m1[:], rcp[:])
                    nc.vector.tensor_add(m1[:], m1[:], cots[b][:])
                    nc.sync.dma_start(out=out[img, 0, r0:r0 + 128, :],
                                      in_=m1[:])

    nc.compile()
    return nc


_NC_CACHE = None


def _get_nc():
    global _NC_CACHE
    if _NC_CACHE is None:
        _NC_CACHE = _build()
    return _NC_CACHE


def run(inputs: dict, trace: bool = False):
    """Run on 8 NeuronCores; returns (out [16,1,256,256], BassKernelResults)."""
    aff = np.ascontiguousarray(np.asarray(inputs["affinity"], dtype=np.float32))
    att = np.ascontiguousarray(np.asarray(inputs["attention"], dtype=np.float32))
    cs = np.ascontiguousarray(
        np.asarray(inputs["current_segmentation"], dtype=np.float32))
    co = np.ascontiguousarray(
        np.asarray(inputs["coarse_segmentation"], dtype=np.float32))
    band = _band_matrix()

    nc = _get_nc()
    in_maps = []
    for c in range(N_CORES):
        s = slice(c * B_CORE, (c + 1) * B_CORE)
        in_maps.append({
            "affinity": np.ascontiguousarray(aff[s]),
            "attention": np.ascontiguousarray(att[s]),
            "current_segmentation": np.ascontiguousarray(cs[s]),
            "coarse_segmentation": np.ascontiguousarray(co[s]),
            "band": band,
        })
    last_err = None
    for attempt in range(3):
        try:
            res = run_bass_kernel_spmd(nc, in_maps, list(range(N_CORES)),
                                       trace=trace)
            break
        except Exception as e:  # transient NRT_EXEC_UNIT_UNRECOVERABLE flakes
            last_err = e
            import time
            time.sleep(10)
    else:
        raise last_err
    full = np.concatenate([res.results[c]["out"] for c in range(N_CORES)], axis=0)
    return full, res


def kernel(**inputs) -> np.ndarray:
    out, _ = run(inputs, trace=False)
    return out

